# revision 25
# baseline (speedup 1.0000x reference)
"""Trainium2 Bass kernel for nn_MessagePackBlock (gnn_message_passing).

Strategy (pure edge-parallel across 8 NeuronCores, E sharded evenly):

Per core, feature-major pipeline for shared-weight matmuls; per-edge
tensor-product mids built edge-major so the local-env scalings (ls, lv)
are per-partition ACT/DVE scalings; the per-edge LinearScaleWithWeights
contraction ("monster") runs on the tensor engine with per-edge-pair
stationary weights in bf16 (fast weight load), consuming the
MLP-generated weights wl produced by a shared-weight matmul.

All static scale constants (1/sqrt(fan_in), TP path norms, lin_out
norms) and the output Linear (Lns/Lnv/Les/Lev) are folded into
host-precomputed weight tensors.
"""
import numpy as np
import ml_dtypes

import concourse.bass as bass
import concourse.bacc as bacc
import concourse.mybir as mybir
import concourse.tile as tile
from concourse import bass_utils
from concourse.masks import make_identity

F32 = mybir.dt.float32
F32R = mybir.dt.float32r
BF16 = mybir.dt.bfloat16
AF = mybir.ActivationFunctionType
ALU = mybir.AluOpType

E_FULL = 30000
N_CORES = 8
E_CORE = E_FULL // N_CORES          # 3750
N_TILE = 512                        # edges per outer tile
N_SUB = 256                        # edges per big-matmul subtile
N_BLK = 128                        # edges per mid/monster block
E_PAD = 4096                       # padded per-core edge count (8 tiles)
E_EFF = 3840                       # edges actually processed (30 blocks)


# ---------------------------------------------------------------- host prep
def _prep_weights(inp):
    f32 = np.float32
    s8 = 1.0 / np.sqrt(f32(8.0))
    s64 = 1.0 / np.sqrt(f32(64.0))
    c = 1.0 / (np.sqrt(f32(64.0)) * np.sqrt(f32(64.0)) * np.sqrt(f32(32.0)))
    alpha = 1.0 / np.sqrt(f32(64.0))
    sq3 = np.sqrt(f32(3.0)).astype(f32)

    W = {}
    W["A1"] = (np.concatenate([inp["Wn_mlp1"], inp["We_mlp1"]], 1) * s8).astype(f32)
    A2 = np.zeros((128, 128), f32)
    A2[0:64, 0:64] = inp["Wn_mlp2"]
    A2[64:128, 64:128] = inp["We_mlp2"]
    W["A2"] = (A2 * s64).astype(f32)

    W["W1n_a"] = (inp["Wn_tp1"][0:32] * alpha).astype(f32)
    W["W1n_b"] = (inp["Wn_tp1"][32:64] * alpha).astype(f32)
    W["W2n_a"] = (inp["Wn_tp2"][0:32] * alpha).astype(f32)
    W["W2n_b"] = (inp["Wn_tp2"][32:64] * alpha).astype(f32)
    alpha_e = 1.0 / np.sqrt(f32(32.0))
    W["W1e"] = (inp["We_tp1"] * alpha_e).astype(f32)
    W["W2e"] = (inp["We_tp2"] * alpha_e).astype(f32)

    def blockdelta(Wmat, ch_off):
        R = np.zeros((96, 96), f32)
        for ch in range(32):
            for m in range(3):
                R[3 * ch + m, m * 32:(m + 1) * 32] = Wmat[ch_off + ch, :]
        return R

    W["R3n_src"] = blockdelta(inp["Wn_tp3"] * alpha, 0)
    W["R3n_dst"] = blockdelta(inp["Wn_tp3"] * alpha, 32)
    W["R4n_src"] = blockdelta(inp["Wn_tp4"] * alpha / sq3, 0)
    W["R4n_dst"] = blockdelta(inp["Wn_tp4"] * alpha / sq3, 32)
    W["R3e"] = blockdelta(inp["We_tp3"] * alpha_e, 0)
    W["R4e"] = blockdelta(inp["We_tp4"] * alpha_e / sq3, 0)

    WB = np.zeros((128, 2, 32, 128), f32)
    for bi, (w3, Ls, Lv) in enumerate(
        [(inp["Wn_mlp3"], inp["Lns"], inp["Lnv"]),
         (inp["We_mlp3"], inp["Les"], inp["Lev"])]
    ):
        w3r = np.asarray(w3, f32).reshape(64, 2, 64, 32)
        k0 = u0 = 64 * bi
        WB[k0:k0 + 64, 0, :, u0:u0 + 64] = np.einsum("kuw,wv->kvu", w3r[:, 0], np.asarray(Ls, f32)) * c
        WB[k0:k0 + 64, 1, :, u0:u0 + 64] = np.einsum("kuw,wv->kvu", w3r[:, 1], np.asarray(Lv, f32)) * c
    W["WB"] = WB.reshape(128, 2 * 32 * 128)

    perm = np.zeros(128, np.int64)
    perm[:32] = np.arange(32)
    for m in range(3):
        for w in range(32):
            perm[32 + 3 * w + m] = 32 + 32 * m + w
    W["perm"] = perm
    return W


SMALL_W = ["W1n_a", "W1n_b", "W2n_a", "W2n_b", "W1e", "W2e"]
R_W = ["R3n_src", "R3n_dst", "R4n_src", "R4n_dst", "R3e", "R4e"]
WEIGHT_NAMES = ["A1", "A2"] + SMALL_W + R_W + ["WB"]


# ---------------------------------------------------------------- bass build
def build_nc(e_pad=E_PAD, monster="pe", wl_dt=BF16, n_passes=1, **kw):
    """Build the per-core SPMD bass program.

    n_passes > 1 repeats the full pipeline (including weight loads) that
    many times inside one program; used only for timing (the marginal
    per-pass time of an R-pass program vs a 1-pass program is pure HW
    execution time, with all per-dispatch overheads cancelled).
    """
    nc = bacc.Bacc("TRN2", target_bir_lowering=False, debug=False)
    skip_out = kw.get("skip_out", False)
    w_reload = kw.get("w_reload", True)

    ins_d = {}
    for nm in ["ns", "nd", "ef"]:
        ins_d[nm + "_s"] = nc.dram_tensor(nm + "_s", [32, e_pad], BF16, kind="ExternalInput").ap()
        ins_d[nm + "_v"] = nc.dram_tensor(nm + "_v", [96, e_pad], BF16, kind="ExternalInput").ap()
    esT = nc.dram_tensor("esT", [8, e_pad], BF16, kind="ExternalInput").ap()
    lee = nc.dram_tensor("lee", [e_pad, 4], F32, kind="ExternalInput").ap()

    dw = {}
    dw["A1"] = nc.dram_tensor("A1", [8, 128], BF16, kind="ExternalInput").ap()
    dw["A2"] = nc.dram_tensor("A2", [128, 128], BF16, kind="ExternalInput").ap()
    for nm in SMALL_W:
        dw[nm] = nc.dram_tensor(nm, [32, 32], BF16, kind="ExternalInput").ap()
    for nm in R_W:
        dw[nm] = nc.dram_tensor(nm, [96, 96], BF16, kind="ExternalInput").ap()
    dw["WB"] = nc.dram_tensor("WB", [128, 2 * 32 * 128], BF16, kind="ExternalInput").ap()

    out_d = nc.dram_tensor("out", [e_pad, 128], BF16, kind="ExternalOutput").ap()

    n_tiles = e_pad // N_TILE

    with tile.TileContext(nc) as tc:
        _emit(tc, nc, ins_d, esT, lee, dw, out_d, n_tiles, monster, wl_dt,
              n_passes, skip_out=skip_out, w_reload=w_reload)
    nc.compile()
    return nc


def _emit(tc, nc, ins_d, esT, lee, dw, out_d, n_tiles, monster, wl_dt,
          n_passes=1, skip_out=False, w_reload=True):
    import contextlib

    def apv(t, off, dims):
        """Manual AP over tile t: keep partition dim, custom free dims."""
        return bass.AP(tensor=t.tensor, offset=t.offset + off,
                       ap=[list(t.ap[0])] + [list(d) for d in dims])

    ctx = contextlib.ExitStack()
    with ctx:
        consts = ctx.enter_context(tc.tile_pool(name="consts", bufs=1))
        wpool = ctx.enter_context(tc.tile_pool(name="wpool", bufs=2))
        inpool = ctx.enter_context(tc.tile_pool(name="inpool", bufs=2))
        mlp = ctx.enter_context(tc.tile_pool(name="mlp", bufs=2))
        wlpool = ctx.enter_context(tc.tile_pool(name="wlpool", bufs=2))
        midp = ctx.enter_context(tc.tile_pool(name="midp", bufs=2))
        outp = ctx.enter_context(tc.tile_pool(name="outp", bufs=2))
        psum = ctx.enter_context(tc.tile_pool(name="psum", bufs=1, space="PSUM"))
        psum_wl = ctx.enter_context(tc.tile_pool(name="psum_wl", bufs=2, space="PSUM"))
        psum_mop = ctx.enter_context(tc.tile_pool(name="psum_mop", bufs=2, space="PSUM"))

        # ---- compile-time constants
        ident = consts.tile([128, 128], F32)
        make_identity(nc, ident)

        def load_weights():
            sw = {}
            for nm in WEIGHT_NAMES:
                t = wpool.tile(list(dw[nm].shape), dw[nm].dtype,
                               name=f"w_{nm}", tag=f"w_{nm}")
                nc.sync.dma_start(out=t, in_=dw[nm])
                sw[nm] = t
            wb_r = sw["WB"].rearrange("k (s w u) -> k s w u", s=2, w=32)
            return sw, wb_r

        if not w_reload:
            sw, wb_r = load_weights()

        for _pass in range(n_passes):
            # weights reloaded per pass so one pass == one complete kernel
            # execution (w_reload=False is a timing-attribution variant)
            if w_reload:
                sw, wb_r = load_weights()

            _emit_pass(tc, nc, ins_d, esT, lee, out_d, n_tiles, wl_dt,
                       sw, wb_r, ident, inpool, mlp, wlpool, midp, outp,
                       psum, psum_wl, psum_mop, apv, skip_out)


def _emit_pass(tc, nc, ins_d, esT, lee, out_d, n_tiles, wl_dt, sw, wb_r,
               ident, inpool, mlp, wlpool, midp, outp, psum, psum_wl,
               psum_mop, apv, skip_out=False):
        # effective edge count: last tile is a half tile so the 3750
        # real edges per core cost 3840 (not 4096) of DMA + compute
        tiles_l = []
        _e0 = 0
        while _e0 < E_EFF:
            _cnt = min(N_TILE, E_EFF - _e0)
            tiles_l.append((_e0, _cnt))
            _e0 += _cnt
        for e0, cnt in tiles_l:
            sl = slice(e0, e0 + cnt)

            it = {}
            for nm in ["ns", "nd", "ef"]:
                it[nm + "_s"] = inpool.tile([32, N_TILE], BF16, name=nm + "_s", tag=nm + "_s")
                it[nm + "_v"] = inpool.tile([96, N_TILE], BF16, name=nm + "_v", tag=nm + "_v")
                nc.sync.dma_start(out=it[nm + "_s"][:, :cnt], in_=ins_d[nm + "_s"][:, sl])
                nc.sync.dma_start(out=it[nm + "_v"][:, :cnt], in_=ins_d[nm + "_v"][:, sl])
            es_t = inpool.tile([8, N_TILE], BF16, tag="es")
            nc.sync.dma_start(out=es_t[:, :cnt], in_=esT[:, sl])
            lv_t = inpool.tile([128, 4 * (N_TILE // N_BLK)], F32, tag="lee")
            for b in range(cnt // N_BLK):
                nc.sync.dma_start(
                    out=lv_t[:, 4 * b:4 * b + 4],
                    in_=lee[e0 + b * N_BLK: e0 + (b + 1) * N_BLK, :])

            # ---------------- MLP: h2 [128, cnt]
            h1p = psum.tile([128, N_TILE], F32, tag="h1p")
            nc.tensor.matmul(h1p[:, :cnt], sw["A1"], es_t[:, :cnt])
            h1 = mlp.tile([128, N_TILE], BF16, tag="h1")
            nc.scalar.activation(h1[:, :cnt], h1p[:, :cnt], AF.Silu)
            h2p = psum.tile([128, N_TILE], F32, tag="h1p")
            nc.tensor.matmul(h2p[:, :cnt], sw["A2"], h1[:, :cnt])
            h2 = mlp.tile([128, N_TILE], BF16, tag="h2")
            nc.scalar.activation(h2[:, :cnt], h2p[:, :cnt], AF.Silu)

            # ---------------- big matmul: wl [128 u', 2 sv, 32 w', N_SUB]
            wl_subs = []
            for s_i in range(cnt // N_SUB):
                c0 = s_i * N_SUB
                wl_sb = wlpool.tile([128, N_SUB, 2, 32], wl_dt, tag="wl")
                for sv in range(2):
                    for wq in range(16):  # pairs of w' per psum tile
                        wlp = psum_wl.tile([128, 2, N_SUB], F32, tag="wlp")
                        for wi in range(2):
                            wp = wq * 2 + wi
                            nc.tensor.matmul(
                                wlp[:, wi],
                                wb_r[:, sv, wp],
                                h2[:, c0:c0 + N_SUB],
                            )
                        wl_dst = wl_sb[:, :, sv, wq * 2:(wq + 1) * 2].transpose([0, 2, 1])
                        if wq % 2 == 0:
                            nc.scalar.activation(wl_dst, wlp, AF.Copy)
                        else:
                            nc.vector.tensor_copy(wl_dst, wlp)
                wl_subs.append(wl_sb)

            # ---------------- per 128-edge block: TP mids + monster
            for b in range(cnt // N_BLK):
                eb = slice(b * N_BLK, (b + 1) * N_BLK)
                lsv = lv_t[:, 4 * b:4 * b + 4]
                ls_ap = lsv[:, 0:1]

                mr = psum.tile([128, 512], F32, tag="midraw")
                # K=32 matmuls: W1/W2 pre-activations
                nc.tensor.matmul(mr[:, 0:32], it["ns_s"][:, eb], sw["W1n_a"], start=True, stop=False)
                nc.tensor.matmul(mr[:, 0:32], it["nd_s"][:, eb], sw["W1n_b"], start=False, stop=True)
                nc.tensor.matmul(mr[:, 32:64], it["ef_s"][:, eb], sw["W1e"])
                nc.tensor.matmul(mr[:, 64:96], it["ns_s"][:, eb], sw["W2n_a"], start=True, stop=False)
                nc.tensor.matmul(mr[:, 64:96], it["nd_s"][:, eb], sw["W2n_b"], start=False, stop=True)
                nc.tensor.matmul(mr[:, 96:128], it["ef_s"][:, eb], sw["W2e"])
                # K=96 block-delta matmuls
                nc.tensor.matmul(mr[:, 128:224], it["ns_v"][:, eb], sw["R3n_src"], start=True, stop=False)
                nc.tensor.matmul(mr[:, 128:224], it["nd_v"][:, eb], sw["R3n_dst"], start=False, stop=True)
                nc.tensor.matmul(mr[:, 224:320], it["ef_v"][:, eb], sw["R3e"])
                nc.tensor.matmul(mr[:, 320:416], it["ns_v"][:, eb], sw["R4n_src"], start=True, stop=False)
                nc.tensor.matmul(mr[:, 320:416], it["nd_v"][:, eb], sw["R4n_dst"], start=False, stop=True)
                nc.tensor.matmul(mr[:, 416:512], it["ef_v"][:, eb], sw["R4e"])

                # mid_sb [128e, 512]: [ms(128) | m0(128) | m1(128) | m2(128)]
                # ms = [W1n*ls | dot_n | W1e*ls | dot_e]
                # mv_m = [W2n*lv_m | o3n_m*ls | W2e*lv_m | o3e_m*ls]
                mid = midp.tile([128, 512], F32, tag="mid")

                # 1) W1 outs * ls -> ms cols {0:32, 64:96}
                nc.scalar.activation(
                    apv(mid, 0, [[64, 2], [1, 32]]),
                    apv(mr, 0, [[32, 2], [1, 32]]),
                    AF.Copy, scale=ls_ap)
                # 2) o3 (g,m,u) * ls -> mv_m cols {32:64, 96:128} of block 1+m
                nc.scalar.activation(
                    apv(mid, 160, [[128, 3], [64, 2], [1, 32]]),
                    apv(mr, 128, [[32, 3], [96, 2], [1, 32]]),
                    AF.Copy, scale=ls_ap)
                # 3) W2 outs * lv_m -> mv_m cols {0:32, 64:96} of block 1+m
                nc.vector.tensor_mul(
                    apv(mid, 128, [[128, 3], [64, 2], [1, 32]]),
                    apv(mr, 64, [[0, 3], [32, 2], [1, 32]]),
                    apv(lv_t, 4 * b + 1, [[1, 3], [0, 2], [0, 32]]))
                # 4) G4 * lv -> dtmp [g,m,u], reduce over m -> ms cols {32:64, 96:128}
                dtmp = midp.tile([128, 2, 3, 32], F32, tag="dtmp")
                nc.vector.tensor_mul(
                    dtmp,
                    apv(mr, 320, [[96, 2], [32, 3], [1, 32]]),
                    apv(lv_t, 4 * b + 1, [[0, 2], [1, 3], [0, 32]]))
                nc.vector.tensor_reduce(
                    apv(mid, 32, [[64, 2], [1, 32]]),
                    dtmp.transpose([0, 1, 3, 2]),
                    mybir.AxisListType.X, ALU.add)

                # ---------------- transpose mids -> M_T [128 u', 4 blk, 128 e]
                mtp = psum.tile([128, 4, 128], F32, tag="mtp")
                for q in range(4):
                    nc.tensor.transpose(mtp[:, q], mid[:, q * 128:(q + 1) * 128], ident)
                m_t = midp.tile([128, 128, 4], wl_dt, tag="m_t")
                nc.scalar.activation(m_t.transpose([0, 2, 1]), mtp, AF.Copy)

                # ---------------- monster: per edge pair
                s_i = b // (N_SUB // N_BLK)
                wl_sb = wl_subs[s_i]
                eoff = (b % (N_SUB // N_BLK)) * N_BLK
                mop = psum_mop.tile([128, 512], F32, tag="mop")
                for p in range(N_BLK // 2):
                    elo = eoff + 2 * p
                    lhsT = wl_sb[:, elo:elo + 2]
                    rhs = m_t[:, 2 * p:2 * p + 2]
                    nc.tensor.matmul(mop[:, 8 * p:8 * p + 8], lhsT, rhs)

                obuf = outp.tile([128, 512], F32, tag="obuf")
                nc.scalar.activation(obuf, mop, AF.Copy)

                # transpose back: 4 blocks of [128,128]
                otp = psum.tile([128, 4, 128], F32, tag="otp")
                for q in range(4):
                    nc.tensor.transpose(otp[:, q], obuf[:, q * 128:(q + 1) * 128], ident)
                osb = outp.tile([128, 4, 128], BF16, tag="osb")
                nc.vector.tensor_copy(osb, otp)

                # out DMA: edge e = 32q + 2k + h; parts at partition 8k+4h+c
                # of q-block; free 64h+32*(c>0)..+32
                if skip_out:
                    continue
                ebase = e0 + b * N_BLK
                for h in range(2):
                    for cpart in range(4):
                        part0 = 4 * h + cpart
                        fr0 = 64 * h + (32 if cpart > 0 else 0)
                        src = osb[part0:128:8, :, fr0:fr0 + 32]
                        dst = bass.AP(
                            tensor=out_d.tensor,
                            offset=out_d.offset + (ebase + h) * 128 + 32 * cpart,
                            ap=[[2 * 128, 16], [32 * 128, 4], [1, 32]])
                        nc.sync.dma_start(out=dst, in_=src)


# ---------------------------------------------------------------- runner
_NC_CACHE = {}


def _get_nc(n_passes=1):
    key = (E_PAD, "pe", n_passes)
    if key not in _NC_CACHE:
        _NC_CACHE[key] = build_nc(n_passes=n_passes)
    return _NC_CACHE[key]


# Persistent PJRT runner: run_bass_kernel_spmd builds a fresh jax.jit
# closure per call (full retrace + XLA/neuronx recompile every time, ~2s).
# Build the sharded executable ONCE and reuse it; repeat calls are then a
# single dispatch.  `chain_k` executions of the bass program are issued
# back-to-back inside one jit so warm-timing the marginal cost of one more
# execution isolates pure on-device time from the ~85ms axon RPC overhead.
_RUNNERS = {}


def _runner_meta(nc):
    import jax
    from concourse import bass2jax

    bass2jax.install_neuronx_cc_hook()
    pname = nc.partition_id_tensor.name if nc.partition_id_tensor else None
    in_names, out_names, out_avals, out_shapes = [], [], [], []
    for alloc in nc.m.functions[0].allocations:
        if not isinstance(alloc, mybir.MemoryLocationSet):
            continue
        name = alloc.memorylocations[0].name
        if alloc.kind == "ExternalInput":
            if name != pname:
                in_names.append(name)
        elif alloc.kind == "ExternalOutput":
            out_names.append(name)
            shape = tuple(alloc.tensor_shape)
            dt = mybir.dt.np(alloc.dtype)
            out_avals.append(jax.core.ShapedArray(shape, dt))
            out_shapes.append((shape, dt))
    return pname, in_names, out_names, out_avals, out_shapes


def _get_runner(chain_k=1, nc=None, cache_key=None):
    key = (chain_k, cache_key)
    if key in _RUNNERS:
        return _RUNNERS[key]
    import jax
    from jax.sharding import Mesh, PartitionSpec
    import warnings
    with warnings.catch_warnings():
        warnings.simplefilter("ignore")
        from jax.experimental.shard_map import shard_map
    from concourse import bass2jax

    if nc is None:
        nc = _get_nc()
    pname, in_names, out_names, out_avals, out_shapes = _runner_meta(nc)
    all_names = list(in_names) + list(out_names) + ([pname] if pname else [])
    n_outs = len(out_names)

    def _exec_once(args):
        operands = list(args)
        if pname is not None:
            operands.append(bass2jax.partition_id_tensor())
        return bass2jax._bass_exec_p.bind(
            *operands,
            out_avals=tuple(out_avals),
            in_names=tuple(all_names),
            out_names=tuple(out_names),
            lowering_input_output_aliases=(),
            sim_require_finite=True,
            sim_require_nnan=True,
            nc=nc,
        )

    def _body(*args):
        outs = []
        for _ in range(chain_k):
            outs.extend(_exec_once(args))
        return tuple(outs)

    devices = jax.devices()[:N_CORES]
    mesh = Mesh(np.asarray(devices), ("core",))
    fn = jax.jit(
        shard_map(
            _body,
            mesh=mesh,
            in_specs=(PartitionSpec("core"),) * (len(in_names) + n_outs),
            out_specs=(PartitionSpec("core"),) * (n_outs * chain_k),
            check_rep=False,
        ),
        keep_unused=True,
    )
    _RUNNERS[key] = (fn, mesh, in_names, out_shapes)
    return _RUNNERS[key]


def _concat_args(in_maps, in_names, out_shapes):
    concat_in = [
        np.concatenate([in_maps[c][nm] for c in range(N_CORES)], axis=0)
        for nm in in_names
    ]
    concat_zeros = [
        np.zeros((N_CORES * s[0], *s[1:]), dt) for (s, dt) in out_shapes
    ]
    return concat_in, concat_zeros


def _postprocess(out_global, W):
    out = np.asarray(out_global).reshape(N_CORES, E_PAD, 128)[:, :E_CORE]
    out = out.reshape(E_FULL, 128)
    return np.ascontiguousarray(out[:, W["perm"]]).astype(np.float32)


def kernel(**inputs):
    inputs = {k: np.asarray(v, np.float32) for k, v in inputs.items()}
    fn, mesh, in_names, out_shapes = _get_runner(chain_k=1)
    in_maps, W = _make_in_maps(inputs)
    concat_in, concat_zeros = _concat_args(in_maps, in_names, out_shapes)
    outs = fn(*concat_in, *concat_zeros)
    return _postprocess(outs[0], W)


def _make_in_maps(inputs):
    W = _prep_weights(inputs)
    in_maps = []
    for c in range(N_CORES):
        sl = slice(c * E_CORE, (c + 1) * E_CORE)
        m = {}

        def padT(x):
            xT = np.zeros((x.shape[1], E_PAD), np.float32)
            xT[:, :E_CORE] = x[sl].T
            return xT

        for nm, key in [("ns", "node_feats_src"), ("nd", "node_feats_dst"), ("ef", "edge_feats")]:
            xT = padT(inputs[key])
            m[nm + "_s"] = np.ascontiguousarray(xT[0:32]).astype(ml_dtypes.bfloat16)
            m[nm + "_v"] = np.ascontiguousarray(xT[32:128]).astype(ml_dtypes.bfloat16)
        m["esT"] = padT(inputs["edge_scalars"]).astype(ml_dtypes.bfloat16)
        le = np.zeros((E_PAD, 4), np.float32)
        le[:E_CORE] = inputs["local_env_edge"][sl]
        m["lee"] = le
        for nm in WEIGHT_NAMES:
            m[nm] = np.ascontiguousarray(W[nm]).astype(ml_dtypes.bfloat16)
        in_maps.append(m)
    return in_maps, W


def timed_run(inputs, n_lo=17, n_hi=33, chain_k=17, rounds=60):
    """Return per-execution on-device time (ns) of the compiled kernel.

    NTFF profiling is not supported on this axon terminal
    (axon_stop_nrt_profile rc=-1 wedges the device), so HW time is
    measured as the marginal on-device cost of one more pipeline pass:
    two program variants unroll the complete per-core pipeline (weight
    loads included, so one pass == one complete kernel execution) n_lo
    and n_hi times inside one program; each is dispatched as chain_k
    chained executions per call.  Both variants are long-running
    programs, so per-execution runtime launch overhead (~0.25ms, which a
    null-kernel calibration shows is independent of kernel content and
    partially hidden behind long executions) is identical on both sides
    and cancels in the difference, as do the ~85ms axon RPC dispatch
    overhead and all host<->device transfers:

        T_pass = (T[n_hi] - T[n_lo]) / (chain_k * (n_hi - n_lo))

    Measurements alternate call order each round and use the median of
    paired differences, cancelling slow drift of the tunnel latency.
    Inputs are device-resident before timing.
    """
    import time as _time
    import jax
    from jax.sharding import NamedSharding, PartitionSpec

    inputs = {k: np.asarray(v, np.float32) for k, v in inputs.items()}
    in_maps, W = _make_in_maps(inputs)

    _, mesh, in_names, out_shapes = _get_runner(chain_k=1)
    fn_lo = _get_runner(chain_k=chain_k, nc=_get_nc(n_passes=n_lo),
                        cache_key=f"p{n_lo}")[0]
    fn_hi = _get_runner(chain_k=chain_k, nc=_get_nc(n_passes=n_hi),
                        cache_key=f"p{n_hi}")[0]
    concat_in, concat_zeros = _concat_args(in_maps, in_names, out_shapes)
    sh = NamedSharding(mesh, PartitionSpec("core"))
    dev_in = [jax.device_put(a, sh) for a in concat_in]
    dev_zeros = [jax.device_put(a, sh) for a in concat_zeros]
    jax.block_until_ready(dev_in + dev_zeros)

    def _call(fn):
        t0 = _time.perf_counter()
        jax.block_until_ready(fn(*dev_in, *dev_zeros))
        return _time.perf_counter() - t0

    _call(fn_lo)  # warm / compile
    _call(fn_hi)
    diffs = []
    for r in range(rounds):
        if r % 2 == 0:
            t_lo = _call(fn_lo)
            t_hi = _call(fn_hi)
        else:
            t_hi = _call(fn_hi)
            t_lo = _call(fn_lo)
        diffs.append(t_hi - t_lo)
    diffs.sort()
    med = diffs[len(diffs) // 2]
    t_pass = med / (chain_k * (n_hi - n_lo))
    return max(1, int(t_pass * 1e9))
